# revision 58
# baseline (speedup 1.0000x reference)
"""Multi-head attention (B=8, S=1024, H=1024, NH=16) on 8 trn2 NeuronCores.

Data-parallel over batch: one batch element per core. Key optimizations
over a straightforward implementation:

- Host-side key compaction: ~50% of keys are masked out (additive -1e32
  -> exp == 0 exactly), so masked key columns of K^T / V^T are gathered
  away on the host and the kernel runs on SK=640 padded keys instead of
  1024. Pad slots keep the -1e32 bias so they contribute exactly 0 to
  both the numerator and the softmax denominator.
- Q/K weights are pre-tiled on the host into [128, HT*S] layout so every
  weight DMA is contiguous 2KB lines (no strided descriptor storms).
- Projections stream: per output tile, an 8-step contraction chain whose
  first matmul only needs the first input tile, so compute starts ~2us
  after launch instead of waiting for all input DMA.
- Attention is software-pipelined one head at a time: scores matmuls
  (64-row PE groups, alternating by head parity), exp on ScalarE with the
  key-mask as per-partition bias, attn @ V_aug (ones-augmented V so the
  same matmul accumulates the softmax denominator in row 64). The
  score/exp stream runs LAG heads ahead of the attn@V stream so the
  ScalarE exp pipe never starves the PE and vice versa.
- 1/denominator via exp(-ln D) on ScalarE (both functions live in one
  activation table set; vector.reciprocal measures ~6.5us).
- Normalization broadcast via a 16-row selector matmul, multiplied into
  O^T incrementally so the output projection can start on tile 0.
"""

import math
from contextlib import ExitStack

import ml_dtypes
import numpy as np

import concourse.bass as bass  # noqa: F401
import concourse.mybir as mybir
import concourse.tile as tile
from concourse import bacc
from concourse.bass_utils import run_bass_kernel_spmd

# Make Exp resolve to the natural_log_exp_and_others activation-table set
# (the only set that also holds Ln). Otherwise every Ln <-> Exp transition
# costs a ~1.3us ACT_TABLE_LOAD on the critical Scalar queue. Set order is
# preserved so act_func_set_id indices stay valid.
_orig_get_tables = bacc.get_activation_tables


def _get_tables_ln_exp(module_arch):
    tables = _orig_get_tables(module_arch)
    exp_t = mybir.ActivationFunctionType.Exp
    return {
        name: (fns - {exp_t} if name != "natural_log_exp_and_others" else fns)
        for name, fns in tables.items()
    }


bacc.get_activation_tables = _get_tables_ln_exp

B, S, H, NH = 8, 1024, 1024, 16
HD = H // NH  # 64
P = 128
HT = H // P  # 8
SK = 640  # compacted+padded key count (max unmasked over batches is 546)
SKT = SK // P  # 5
VA = HD + 1  # 65 (ones-augmented head dim)
NI = 512
LAG = 5  # score/exp stream runs this many heads ahead of attn@V
AT_BUFS = 5 * LAG + 3
NEG = np.float32(-1e32)
SCALE = 1.0 / math.sqrt(H)

XSC = 16.0  # fp8 scale for Q/K projection inputs
WSC = 64.0  # fp8 scale for Wq/Wk (std 0.02 otherwise lands subnormal)
QSC = XSC * WSC  # QT/KTc carry this factor; folded into the exp scale

BF = mybir.dt.bfloat16
F32 = mybir.dt.float32
FP8 = mybir.dt.float8e4
EXP = mybir.ActivationFunctionType.Exp
LN = mybir.ActivationFunctionType.Ln
DR = mybir.MatmulPerfMode.DoubleRow

_CACHE: dict = {}


def build_program():
    nc = bacc.Bacc(None, target_bir_lowering=False)

    xq2_d = nc.declare_dram_parameter("xq2", [H // 2, 2 * S], FP8, isOutput=False)
    xk2_d = nc.declare_dram_parameter("xk2", [H // 2, 2 * SK], FP8, isOutput=False)
    xvc_d = nc.declare_dram_parameter("xvc", [H, SK], BF, isOutput=False)
    wqL_d = nc.declare_dram_parameter("wqL", [P, HT * S], FP8, isOutput=False)
    wkL_d = nc.declare_dram_parameter("wkL", [P, HT * S], FP8, isOutput=False)
    wvR_d = nc.declare_dram_parameter("wvR", [H, H], BF, isOutput=False)
    woR_d = nc.declare_dram_parameter("woR", [H, H], BF, isOutput=False)
    maskc_d = nc.declare_dram_parameter("maskc", [P, SKT], F32, isOutput=False)
    bqT_d = nc.declare_dram_parameter("bqT", [P, HT], F32, isOutput=False)
    bkT_d = nc.declare_dram_parameter("bkT", [P, HT], F32, isOutput=False)
    bvb_d = nc.declare_dram_parameter("bvb", [P, H], BF, isOutput=False)
    bob_d = nc.declare_dram_parameter("bob", [P, H], F32, isOutput=False)
    sel_d = nc.declare_dram_parameter("sel", [NH // 2, 2 * H], BF, isOutput=False)
    y_d = nc.declare_dram_parameter("y", [S, H], F32, isOutput=True)

    with tile.TileContext(nc) as tc, ExitStack() as ctx:
        sb = ctx.enter_context(tc.tile_pool(name="sb", bufs=1))

        # (constants are DMA'd after the first critical Q-proj loads below)

        QT = [sb.tile([P, S], BF, tag=f"QT{i}", name=f"QT{i}") for i in range(HT)]
        KTc = [sb.tile([P, SK], BF, tag=f"KT{i}", name=f"KT{i}") for i in range(HT)]
        Vaug = [
            sb.tile([P, NH * VA], BF, tag=f"Va{j}", name=f"Va{j}") for j in range(SKT)
        ]
        OT = [sb.tile([P, S], BF, tag=f"OT{i}", name=f"OT{i}") for i in range(HT)]
        # Denominators split across two partition-aligned [8, S] tiles
        # (heads 0-7 / 8-15) so 1/D for the first half can be computed as
        # soon as head 7 completes (ACT partition offsets must be aligned).
        DNb = [sb.tile([NH // 2, S], BF, tag=f"DNb{i}", name=f"DNb{i}") for i in range(2)]

        def load_rows(pool, dram, tagp, ncols):
            ts = []
            for kt in range(HT):
                t = pool.tile([P, ncols], BF, tag=f"{tagp}{kt}", name=f"{tagp}{kt}")
                nc.sync.dma_start(out=t[:], in_=dram[kt * P : (kt + 1) * P, :])
                ts.append(t)
            return ts

        # ---------- phase A: Q / K projections (fp8 DoubleRow) ----------
        def load_pairs(pool, dram, tagp, ncols2):
            ts = []
            for t in range(HT // 2):
                tl = pool.tile([P, ncols2], FP8, tag=f"{tagp}{t}", name=f"{tagp}{t}")
                nc.sync.dma_start(out=tl[:], in_=dram[t * P : (t + 1) * P, :])
                ts.append(tl)
            return ts

        # Warm the PE clock gate (HAM) on zeros during the input-DMA wait:
        # ~10 matmuls of sustained activity lift K/N to 8/8 before real work.
        wrm = sb.tile([P, NI], BF, tag="wrm")
        nc.vector.memset(wrm[:], 0.0)
        wup = tc.alloc_tile_pool(name="wup", bufs=1, space="PSUM")
        wt = wup.tile([P, NI], F32, tag="wt", name="wt")
        for _ in range(10):
            nc.tensor.matmul(wt[:], wrm[:, 0:P], wrm[:], start=True, stop=True)
        wup.release()

        # Long-lived pools allocated up front (released LIFO at the end).
        atp = tc.alloc_tile_pool(name="atp", bufs=AT_BUFS)
        scp = tc.alloc_tile_pool(name="scp", bufs=2, space="PSUM")
        xvp = tc.alloc_tile_pool(name="xvp", bufs=1)

        # DMA queues have no priority: emission order IS the priority order.
        # xq2[0] + wq block 0 (first matmul's deps) go absolutely first,
        # then everything else in consumption order.
        xqp = tc.alloc_tile_pool(name="xqp", bufs=1)
        wqp = tc.alloc_tile_pool(name="wqp", bufs=1)
        xq2_first = xqp.tile([P, 2 * S], FP8, tag="xq0", name="xq0")
        nc.sync.dma_start(out=xq2_first[:], in_=xq2_d[0:P, :])
        w0_first = wqp.tile([P, S], FP8, tag="wq0", name="wq0")
        nc.sync.dma_start(out=w0_first[:], in_=wqL_d[:, 0:S])

        def load_wblocks(wL_d, tagp, first=None):
            ts = [] if first is None else [first]
            for ot in range(len(ts), HT):
                w = wqp.tile([P, S], FP8, tag=f"{tagp}{ot}", name=f"{tagp}{ot}")
                nc.sync.dma_start(out=w[:], in_=wL_d[:, ot * S : (ot + 1) * S])
                ts.append(w)
            return ts

        wq = load_wblocks(wqL_d, "wq", first=w0_first)
        xq2 = [xq2_first]
        for t in range(1, HT // 2):
            tl = xqp.tile([P, 2 * S], FP8, tag=f"xq{t}", name=f"xq{t}")
            nc.sync.dma_start(out=tl[:], in_=xq2_d[t * P : (t + 1) * P, :])
            xq2.append(tl)
        maskc = sb.tile([P, SKT], F32, tag="maskc")
        nc.sync.dma_start(out=maskc[:], in_=maskc_d[:])
        bqT = sb.tile([P, HT], F32, tag="bqT")
        nc.sync.dma_start(out=bqT[:], in_=bqT_d[:])
        bkT = sb.tile([P, HT], F32, tag="bkT")
        nc.sync.dma_start(out=bkT[:], in_=bkT_d[:])
        xkp = tc.alloc_tile_pool(name="xkp", bufs=1)
        xk2 = load_pairs(xkp, xk2_d, "xk", 2 * SK)
        wk = load_wblocks(wkL_d, "wk")
        # V inputs last among phase-A transfers (needed only in phase B).
        xvc = load_rows(xvp, xvc_d, "xv", SK)
        wv = load_rows(xvp, wvR_d, "wv", S)
        bvb = sb.tile([P, H], BF, tag="bvb")
        nc.sync.dma_start(out=bvb[:], in_=bvb_d[:])
        prjp = tc.alloc_tile_pool(name="prjp", bufs=2, space="PSUM")

        def proj_qk(w_tiles, x2_tiles, out_tiles, bias_tile, ncols):
            for ot in range(HT):
                w3 = w_tiles[ot].rearrange("p (k c) -> p k c", c=P)
                pj = prjp.tile([P, S], F32, tag="pj", name="pj")
                for k2 in range(HT // 2):
                    lhs = w3[:, 2 * k2 : 2 * k2 + 2, :]
                    x3 = x2_tiles[k2].rearrange("p (two c) -> p two c", two=2)
                    nc.tensor.matmul(
                        pj[:, 0:NI], lhs, x3[:, :, 0:NI],
                        start=(k2 == 0), stop=(k2 == HT // 2 - 1), perf_mode=DR,
                    )
                    nc.tensor.matmul(
                        pj[:, NI:ncols], lhs, x3[:, :, NI:ncols],
                        start=(k2 == 0), stop=(k2 == HT // 2 - 1), perf_mode=DR,
                    )
                nc.vector.tensor_scalar_add(
                    out_tiles[ot][:, 0:ncols], pj[:, 0:ncols],
                    bias_tile[:, ot : ot + 1],
                )

        proj_qk(wq, xq2, QT, bqT, S)
        proj_qk(wk, xk2, KTc, bkT, SK)
        prjp.release()
        xkp.release()
        wqp.release()
        xqp.release()

        # ---------- phase B: V projection + early scores/exp ----------
        pvp = tc.alloc_tile_pool(name="pvp", bufs=2, space="PSUM")

        at_map = {}

        def sc_exp(h, jt):
            kt, r0 = h // 2, (h % 2) * HD
            sc = scp.tile([P, S], F32, tag="sc", name="sc")
            lhs = KTc[kt][r0 : r0 + HD, jt * P : (jt + 1) * P]
            nc.tensor.matmul(
                sc[:, 0:NI], lhs, QT[kt][r0 : r0 + HD, 0:NI], start=True, stop=True
            )
            nc.tensor.matmul(
                sc[:, NI:S], lhs, QT[kt][r0 : r0 + HD, NI:S], start=True, stop=True
            )
            at = atp.tile([P, S], BF, tag="at", name="at")
            nc.scalar.activation(
                at[:], sc[:], EXP,
                bias=maskc[:, jt : jt + 1], scale=SCALE / (QSC * QSC),
            )
            at_map[(h, jt)] = at

        # Early score/exp pairs are woven INTO the V-projection chains so
        # the ScalarE exp stream starts immediately and ramps smoothly.
        bq_pend = [(h, jt) for jt in range(SKT) for h in range(LAG)]
        bq_i = 0
        for jt in range(SKT):
            pv = pvp.tile([P, S], F32, tag="pv", name="pv")
            for kt in range(HT):
                lhs = xvc[kt][:, jt * P : (jt + 1) * P]
                nc.tensor.matmul(
                    pv[:, 0:NI], lhs, wv[kt][:, 0:NI],
                    start=(kt == 0), stop=(kt == HT - 1),
                )
                nc.tensor.matmul(
                    pv[:, NI:S], lhs, wv[kt][:, NI:S],
                    start=(kt == 0), stop=(kt == HT - 1),
                )
                if (kt % 2 == 1 or kt == 6) and bq_i < len(bq_pend):
                    sc_exp(*bq_pend[bq_i])
                    bq_i += 1
            va3 = Vaug[jt].rearrange("p (h c) -> p h c", c=VA)
            nc.vector.memset(va3[:, :, HD : HD + 1], 1.0)
            nc.vector.tensor_add(
                va3[:, :, 0:HD],
                pv[:].rearrange("p (h c) -> p h c", c=HD),
                bvb[:].rearrange("p (h c) -> p h c", c=HD),
            )
        pvp.release()
        xvp.release()

        # ---------- phase C: attention heads, software-pipelined ----------
        wop = tc.alloc_tile_pool(name="wop", bufs=1)
        wo = load_rows(wop, woR_d, "wo", S)
        bob = sb.tile([P, H], F32, tag="bob")
        nc.sync.dma_start(out=bob[:], in_=bob_d[:])
        sel = sb.tile([NH // 2, 2 * H], BF, tag="sel")
        nc.sync.dma_start(out=sel[:], in_=sel_d[:])
        avp = tc.alloc_tile_pool(name="avp", bufs=2, space="PSUM")

        lnD = [sb.tile([NH // 2, S], F32, tag=f"lnD{i}", name=f"lnD{i}") for i in range(2)]
        RCb = [sb.tile([NH // 2, S], BF, tag=f"RCb{i}", name=f"RCb{i}") for i in range(2)]
        RC0 = sb.tile([NH // 2, S], F32, tag="RC0")

        def recip_den(i):
            # 1/D = exp(-ln D) on ScalarE (table set shared with Exp).
            nc.scalar.activation(lnD[i][:], DNb[i][:], LN)
            nc.scalar.activation(RCb[i][:], lnD[i][:], EXP, scale=-1.0)

        def rt_mul(kt):
            half = kt // 4
            rt = scp.tile([P, S], F32, tag="sc", name="rt")
            for ic in range(2):
                cc = slice(ic * NI, (ic + 1) * NI)
                nc.tensor.matmul(
                    rt[:, cc],
                    sel[:, half * H + kt * P : half * H + (kt + 1) * P],
                    RCb[half][:, cc],
                    start=True, stop=True,
                )
            # Halved so the first py chains unblock after ~0.65us, not 1.2us
            nc.vector.tensor_mul(OT[kt][:, 0:NI], OT[kt][:, 0:NI], rt[:, 0:NI])
            nc.vector.tensor_mul(OT[kt][:, NI:S], OT[kt][:, NI:S], rt[:, NI:S])

        for h in range(NH):
            hs = h + LAG
            av = avp.tile([VA, S], F32, tag="av", name="av")
            for jt in range(SKT):
                if hs < NH:
                    sc_exp(hs, jt)
                at = at_map.pop((h, jt))
                lhs = Vaug[jt][:, h * VA : (h + 1) * VA]
                nc.tensor.matmul(
                    av[:, 0:NI], lhs, at[:, 0:NI],
                    start=(jt == 0), stop=(jt == SKT - 1),
                )
                nc.tensor.matmul(
                    av[:, NI:S], lhs, at[:, NI:S],
                    start=(jt == 0), stop=(jt == SKT - 1),
                )
            st = sb.tile([VA, S], BF, tag="stage", bufs=2, name="stage")
            nc.vector.tensor_copy(st[:], av[:])
            kt, r0 = h // 2, (h % 2) * HD
            nc.sync.dma_start(out=OT[kt][r0 : r0 + HD, :], in_=st[0:HD, :])
            nc.sync.dma_start(
                out=DNb[h // 8][h % 8 : h % 8 + 1, :], in_=st[HD : HD + 1, :]
            )
            # First-half reciprocals on the (slack) Vector engine mid-C,
            # split in column halves so no single insertion delays the
            # stage-copy chain by more than ~3.3us. RCb[0] is then ready
            # well before phase D — only the last head pair's reciprocal
            # (recip_den(1), ScalarE) gates the tail.
            if h == NH // 2 - 1:
                nc.vector.reciprocal(RC0[:, 0:NI], DNb[0][:, 0:NI])
            elif h == NH // 2:
                nc.vector.reciprocal(RC0[:, NI:S], DNb[0][:, NI:S])
            elif h == NH // 2 + 1:
                nc.vector.tensor_copy(RCb[0][:], RC0[:])
        recip_den(1)

        def warm_fill(n):
            # Dependency-free dummy matmuls: bridge short PE stalls in the
            # recip-gated endgame so HAM never re-throttles the clock.
            for _ in range(n):
                d = scp.tile([P, S], F32, tag="sc", name="dwarm")
                nc.tensor.matmul(
                    d[:, 0:NI], wrm[:, 0:P], wrm[:], start=True, stop=True
                )

        warm_fill(4)
        avp.release()

        # ---------- phase D: normalization + output projection ----------
        opj = tc.alloc_tile_pool(name="opj", bufs=2, space="PSUM")

        def py_chain(py, st_, kts, start0, stop7):
            for kt in kts:
                lhs = OT[kt][:, st_ * P : (st_ + 1) * P]
                nc.tensor.matmul(
                    py[:, 0:NI], lhs, wo[kt][:, 0:NI],
                    start=(kt == 0), stop=(kt == HT - 1),
                )
                nc.tensor.matmul(
                    py[:, NI:S], lhs, wo[kt][:, NI:S],
                    start=(kt == 0), stop=(kt == HT - 1),
                )

        def py_finish(py, st_):
            ysb = sb.tile([P, S], F32, tag="ysb", bufs=2, name="ysb")
            nc.vector.tensor_add(ysb[:], py[:], bob[:])
            nc.sync.dma_start(out=y_d[st_ * P : (st_ + 1) * P, :], in_=ysb[:])

        # warm_fill(1) between steps bridges the rt->mul->py latency gaps
        # (~1us each) that otherwise flip the PE clock gate cold for the
        # whole output projection.
        for kt in range(4):
            rt_mul(kt)
            warm_fill(1)
        py0 = opj.tile([P, S], F32, tag="py", name="py0")
        py_chain(py0, 0, range(4), True, False)
        warm_fill(1)
        py1 = opj.tile([P, S], F32, tag="py", name="py1")
        py_chain(py1, 1, range(4), True, False)
        warm_fill(1)
        for kt in range(4, HT):
            rt_mul(kt)
            warm_fill(1)
        py_chain(py0, 0, range(4, HT), False, True)
        py_finish(py0, 0)
        py_chain(py1, 1, range(4, HT), False, True)
        py_finish(py1, 1)
        for st_ in range(2, HT):
            py = opj.tile([P, S], F32, tag="py", name="py")
            py_chain(py, st_, range(HT), True, True)
            py_finish(py, st_)
        opj.release()
        wop.release()
        scp.release()
        atp.release()

    nc.compile()
    return nc


def _bf(x):
    return np.ascontiguousarray(np.asarray(x, np.float32), dtype=ml_dtypes.bfloat16)


def _f8(x):
    return np.ascontiguousarray(
        np.asarray(x, np.float32), dtype=ml_dtypes.float8_e4m3fn
    )


def _f32(x):
    return np.ascontiguousarray(x, dtype=np.float32)


def _pair_rows(xT, ncols):
    """[H, ncols] -> [H//2, 2*ncols]: row-tile pairs (2t, 2t+1) side by side,
    so a DoubleRow rhs AP [P, 2, ncols] covers two 128-row k-tiles."""
    t = xT.reshape(HT, P, ncols)
    out = np.empty((HT // 2, P, 2 * ncols), xT.dtype)
    for i in range(HT // 2):
        out[i, :, :ncols] = t[2 * i]
        out[i, :, ncols:] = t[2 * i + 1]
    return out.reshape(H // 2, 2 * ncols)


def _tile_wL(wT):
    """[H, H] (f, fout) -> [P, HT*S]: per output-block ot a [P, S] tile whose
    (p, kt*P + c) element is wT[kt*P + p, ot*P + c]."""
    blocks = []
    for ot in range(HT):
        blk = wT[:, ot * P : (ot + 1) * P]  # [H, P]
        blocks.append(
            blk.reshape(HT, P, P).transpose(1, 0, 2).reshape(P, H)
        )
    return np.concatenate(blocks, axis=1)  # [P, HT*H]


def prep_inputs(query, key, value, mask, Wq, bq, Wk, bk, Wv, bv, Wo, bo):
    """Build the 8 per-core input maps (host-side sharding + layout prep)."""
    wqL = _f8(_tile_wL(np.asarray(Wq, np.float32).T) * WSC)
    wkL = _f8(_tile_wL(np.asarray(Wk, np.float32).T) * WSC)
    wvR = _bf(np.asarray(Wv, np.float32).T)
    woR = _bf(np.asarray(Wo, np.float32).T)
    bqT = _f32(np.asarray(bq, np.float32).reshape(HT, P).T * QSC)
    bkT = _f32(np.asarray(bk, np.float32).reshape(HT, P).T * QSC)
    bvb = _bf(np.broadcast_to(np.asarray(bv, np.float32), (P, H)))
    bob = _f32(np.broadcast_to(np.asarray(bo, np.float32), (P, H)))
    # sel[hl, half*H + f] = 1 iff feature f belongs to head 8*half + hl
    selm = np.zeros((NH // 2, 2 * H), np.float32)
    cols = np.arange(H)
    for half in range(2):
        heads = cols // HD
        rows = heads - 8 * half
        valid = (rows >= 0) & (rows < NH // 2)
        selm[rows[valid], half * H + cols[valid]] = 1.0
    selm = _bf(selm)

    in_maps = []
    for b in range(B):
        mb = np.asarray(mask[b])
        keep = np.flatnonzero(~mb)
        n = len(keep)
        assert n <= SK, f"batch {b}: {n} unmasked keys > SK={SK}"
        xkT = np.asarray(key[b], np.float32).T
        xvT = np.asarray(value[b], np.float32).T
        xkc = np.zeros((H, SK), np.float32)
        xkc[:, :n] = xkT[:, keep]
        xvc = np.zeros((H, SK), np.float32)
        xvc[:, :n] = xvT[:, keep]
        maskcol = np.full(SK, NEG, np.float32)
        maskcol[:n] = 0.0
        in_maps.append(
            {
                "xq2": _f8(
                    _pair_rows(np.asarray(query[b], np.float32).T, S) * XSC
                ),
                "xk2": _f8(_pair_rows(xkc, SK) * XSC),
                "xvc": _bf(xvc),
                "wqL": wqL,
                "wkL": wkL,
                "wvR": wvR,
                "woR": woR,
                "maskc": _f32(maskcol.reshape(SKT, P).T),
                "bqT": bqT,
                "bkT": bkT,
                "bvb": bvb,
                "bob": bob,
                "sel": selm,
            }
        )
    return in_maps


def kernel(
    query, key, value, mask, seq_mask, Wq, bq, Wk, bk, Wv, bv, Wo, bo, **run_kwargs
):
    assert int(np.asarray(seq_mask)) == 0, "causal masking not implemented"
    if "nc" not in _CACHE:
        _CACHE["nc"] = build_program()
    nc = _CACHE["nc"]
    in_maps = prep_inputs(query, key, value, mask, Wq, bq, Wk, bk, Wv, bv, Wo, bo)
    res = run_bass_kernel_spmd(nc, in_maps, list(range(B)), **run_kwargs)
    out = np.stack([res.results[b]["y"] for b in range(B)], axis=0)
    if run_kwargs:
        _CACHE["last_result"] = res
    return out
